# revision 6
# baseline (speedup 1.0000x reference)
"""Bahdanau attention forward on 8 Trainium2 NeuronCores (Bass/Tile).

Data-parallel: batch 32 sharded 4-per-core; weights replicated. Per core:
  enc_projT[d, s] = sum_e W_enc[e, d] * X[b, s, e]        (bf16 matmul, fp32 acc)
  energyT = tanh(enc_projT + dec_projT[d, b])             (ACT, per-partition bias)
  scores[s] = sum_d v[d] * energyT[d, s]                  (bf16 matmul)
  weights = softmax(scores + maskneg)                     (DVE/ACT, partition 0)
  context[e] = sum_s weights[s] * X[b, s, e]              (bf16 matmul)

Layouts: X arrives fp32; a gpsimd cast-DMA makes a natural bf16 copy in SBUF,
and an HWDGE x-bar transpose DMA (SBUF->SBUF, bf16) produces X^T tiles for the
enc_proj matmul. W_enc/W_dec/v/h are pre-cast/transposed host-side (tiny,
replicated). The softmax weight vector bounces through a DRAM scratch tile to
re-layout [1, S] -> [128, S/128] with a bf16 cast for the context matmul.
"""

from contextlib import ExitStack

import numpy as np
import ml_dtypes

import concourse.bass as bass  # noqa: F401  (Bass types via bacc)
import concourse.mybir as mybir
import concourse.tile as tile
from concourse import bacc
from concourse.bass_utils import run_bass_kernel_spmd

F32 = mybir.dt.float32
BF16 = mybir.dt.bfloat16
AFT = mybir.ActivationFunctionType

B, S, E, D = 32, 2048, 1024, 1024
N_CORES = 8
BL = B // N_CORES


def build_program(BL=4, S=2048, E=1024, D=1024, schunk=512):
    """Build and compile the per-core SPMD program. Returns nc."""
    P = 128
    ST = S // P               # s tiles
    SC = S // schunk          # score chunks
    STC = schunk // P         # s tiles per chunk
    ET = E // P
    DT = D // P
    ECH = 512                 # context matmul free-dim chunk
    EC = (E + ECH - 1) // ECH

    nc = bacc.Bacc("TRN2", target_bir_lowering=False, debug=False)

    Xd = nc.declare_dram_parameter("x", [BL, S, E], F32, isOutput=False)
    HTd = nc.declare_dram_parameter("ht", [E, BL], BF16, isOutput=False)
    MNd = nc.declare_dram_parameter("maskneg", [BL, S], F32, isOutput=False)
    WEd = nc.declare_dram_parameter("wenc", [E, D], BF16, isOutput=False)
    WDd = nc.declare_dram_parameter("wdec", [E, D], BF16, isOutput=False)
    VTd = nc.declare_dram_parameter("vt", [P, DT], BF16, isOutput=False)
    CTXd = nc.declare_dram_parameter("context", [BL, E], F32, isOutput=True)
    WTSd = nc.declare_dram_parameter("weights", [BL, S], F32, isOutput=True)

    with tile.TileContext(nc) as tc, ExitStack() as ctx:
        consts = ctx.enter_context(tc.tile_pool(name="consts", bufs=1))
        xn_pool = ctx.enter_context(tc.tile_pool(name="xn", bufs=2))
        xt_pool = ctx.enter_context(tc.tile_pool(name="xt", bufs=4))
        work = ctx.enter_context(tc.tile_pool(name="work", bufs=3))
        wd_pool = ctx.enter_context(tc.tile_pool(name="wd", bufs=2))
        sm_pool = ctx.enter_context(tc.tile_pool(name="sm", bufs=2))
        dram_pool = ctx.enter_context(tc.tile_pool(name="dram", bufs=2, space="DRAM"))
        ps_e_pool = ctx.enter_context(tc.tile_pool(name="pse", bufs=2, space="PSUM"))
        ps_s_pool = ctx.enter_context(tc.tile_pool(name="pss", bufs=2, space="PSUM"))
        ps_c_pool = ctx.enter_context(tc.tile_pool(name="psc", bufs=1, space="PSUM"))
        ps_d_pool = ctx.enter_context(tc.tile_pool(name="psd", bufs=1, space="PSUM"))

        # ---- constants ----
        wenc_sb = consts.tile([P, ET, D], BF16)
        nc.sync.dma_start(
            out=wenc_sb, in_=WEd.ap().rearrange("(et p) d -> p et d", p=P)
        )
        vt_sb = consts.tile([P, DT], BF16)
        nc.sync.dma_start(out=vt_sb, in_=VTd.ap())
        ht_sb = consts.tile([P, ET, BL], BF16)
        nc.sync.dma_start(
            out=ht_sb, in_=HTd.ap().rearrange("(et p) b -> p et b", p=P)
        )

        # ---- dec_projT[d, b] = sum_e W_dec[e, d] h[b, e]  (bf16) ----
        decT_sb = consts.tile([P, DT, BL], F32)
        for dt_i in range(DT):
            # W_dec column strip [e, dt_i*128:(dt_i+1)*128] as et x [128, 128]
            wd_sb = wd_pool.tile([P, ET, P], BF16, tag="wdec")
            nc.sync.dma_start(
                out=wd_sb,
                in_=WDd.ap()[:, dt_i * P : (dt_i + 1) * P].rearrange(
                    "(et p) c -> p et c", p=P
                ),
            )
            ps_d = ps_d_pool.tile([P, BL], F32, tag="psd")
            for et in range(ET):
                nc.tensor.matmul(
                    ps_d,
                    lhsT=wd_sb[:, et, :],
                    rhs=ht_sb[:, et, :],
                    start=(et == 0),
                    stop=(et == ET - 1),
                )
            nc.scalar.copy(out=decT_sb[:, dt_i, :], in_=ps_d)

        # ---- per-batch pipeline ----
        for b in range(BL):
            # natural bf16 copy of X[b]: xn[sp, st, e] = X[b, st*128+sp, e]
            xn_sb = xn_pool.tile([P, ST, E], BF16, tag="xn")
            xin = Xd.ap()[b].rearrange("(st p) e -> p st e", p=P)

            mn_sb = sm_pool.tile([1, S], F32, tag="mn")
            nc.sync.dma_start(out=mn_sb, in_=MNd.ap()[b : b + 1, :])
            scores_sb = sm_pool.tile([1, S], F32, tag="scores")
            for sc in range(SC):
                nc.gpsimd.dma_start(
                    out=xn_sb[:, sc * STC : (sc + 1) * STC, :],
                    in_=xin[:, sc * STC : (sc + 1) * STC, :],
                )
                # transposed chunk: xt[ep, stc, et, sp]
                #   = X[b, (sc*STC+stc)*128+sp, et*128+ep]
                xt_sb = xt_pool.tile([P, STC, ET, P], BF16, tag="xt")
                nc.sync.dma_start(
                    out=xt_sb,
                    in_=xn_sb[:, sc * STC : (sc + 1) * STC, :],
                    transpose=True,
                )
                ps_s = ps_s_pool.tile([1, schunk], F32, tag="pss")
                for dt_i in range(DT):
                    ps_e = ps_e_pool.tile([P, schunk], F32, tag="pse")
                    for et in range(ET):
                        nc.tensor.matmul(
                            ps_e,
                            lhsT=wenc_sb[:, et, dt_i * P : (dt_i + 1) * P],
                            rhs=xt_sb[:, :, et, :],
                            start=(et == 0),
                            stop=(et == ET - 1),
                        )
                    en_sb = work.tile([P, schunk], BF16, tag="energy")
                    nc.scalar.activation(
                        out=en_sb,
                        in_=ps_e,
                        func=AFT.Tanh,
                        bias=decT_sb[:, dt_i, b : b + 1],
                    )
                    nc.tensor.matmul(
                        ps_s,
                        lhsT=vt_sb[:, dt_i : dt_i + 1],
                        rhs=en_sb,
                        start=(dt_i == 0),
                        stop=(dt_i == DT - 1),
                    )
                # psum -> scores row (with additive mask fold-in)
                nc.vector.tensor_add(
                    out=scores_sb[:, sc * schunk : (sc + 1) * schunk],
                    in0=ps_s,
                    in1=mn_sb[:, sc * schunk : (sc + 1) * schunk],
                )

            # softmax over s on partition 0 (in-place on scores_sb)
            negmax = sm_pool.tile([1, 1], F32, tag="negmax")
            nc.vector.tensor_reduce(
                out=negmax,
                in_=scores_sb,
                axis=mybir.AxisListType.X,
                op=mybir.AluOpType.max,
                negate=True,
            )
            ssum = sm_pool.tile([1, 1], F32, tag="ssum")
            nc.scalar.activation(
                out=scores_sb,
                in_=scores_sb,
                func=AFT.Exp,
                bias=negmax,
                accum_out=ssum,
            )
            rinv = sm_pool.tile([1, 1], F32, tag="rinv")
            nc.vector.reciprocal(out=rinv, in_=ssum)

            # normalize in place -> weights row; store + DRAM bounce for w^T
            nc.vector.tensor_scalar_mul(scores_sb, scores_sb, rinv)
            nc.sync.dma_start(out=WTSd.ap()[b : b + 1, :], in_=scores_sb)
            wq_dram = dram_pool.tile([S], F32, tag="wq")
            nc.sync.dma_start(out=wq_dram, in_=scores_sb)
            wt_sb = sm_pool.tile([P, ST], BF16, tag="wt")
            nc.gpsimd.dma_start(
                out=wt_sb, in_=wq_dram[:].rearrange("(st sp) -> sp st", sp=P)
            )

            # context: ctx[e] = sum_s w[s] X[b, s, e]
            ps_c = ps_c_pool.tile([1, E], F32, tag="psc")
            for st in range(ST):
                for ec2 in range(EC):
                    nc.tensor.matmul(
                        ps_c[:, ec2 * ECH : (ec2 + 1) * ECH],
                        lhsT=wt_sb[:, st : st + 1],
                        rhs=xn_sb[:, st, ec2 * ECH : (ec2 + 1) * ECH],
                        start=(st == 0),
                        stop=(st == ST - 1),
                    )
            ctx_sb = sm_pool.tile([1, E], F32, tag="ctx")
            nc.scalar.copy(out=ctx_sb, in_=ps_c)
            nc.sync.dma_start(out=CTXd.ap()[b : b + 1, :], in_=ctx_sb)

    nc.compile()
    return nc


def host_inputs(decoder_hidden, encoder_outputs, mask, W_enc, W_dec, v,
                n_cores=N_CORES):
    """Shard + prep host-side numpy inputs; returns per-core input maps."""
    nb = decoder_hidden.shape[0]
    bl = nb // n_cores
    maskneg = np.where(mask, np.float32(-1e30), np.float32(0.0)).astype(np.float32)
    wenc_b = np.ascontiguousarray(W_enc).astype(ml_dtypes.bfloat16)
    wdec = np.ascontiguousarray(W_dec).astype(ml_dtypes.bfloat16)
    dt_n = v.shape[0] // 128
    vt = np.ascontiguousarray(v.astype(ml_dtypes.bfloat16).reshape(dt_n, 128).T)
    hT = np.ascontiguousarray(decoder_hidden.T).astype(ml_dtypes.bfloat16)
    in_maps = []
    for c in range(n_cores):
        sl = slice(c * bl, (c + 1) * bl)
        in_maps.append(
            {
                "x": np.ascontiguousarray(encoder_outputs[sl]).astype(np.float32),
                "ht": np.ascontiguousarray(hT[:, sl]),
                "maskneg": np.ascontiguousarray(maskneg[sl]),
                "wenc": wenc_b,
                "wdec": wdec,
                "vt": vt,
            }
        )
    return in_maps


_CACHE = {}


def _get_program():
    if "nc" not in _CACHE:
        _CACHE["nc"] = build_program(BL=BL, S=S, E=E, D=D)
    return _CACHE["nc"]


def run(inputs, trace=False):
    """inputs: dict as from setup_inputs(); returns (results, context, weights)."""
    nc = _get_program()
    in_maps = host_inputs(
        np.asarray(inputs["decoder_hidden"], dtype=np.float32),
        np.asarray(inputs["encoder_outputs"], dtype=np.float32),
        np.asarray(inputs["mask"]),
        np.asarray(inputs["W_enc"], dtype=np.float32),
        np.asarray(inputs["W_dec"], dtype=np.float32),
        np.asarray(inputs["v"], dtype=np.float32),
    )
    res = run_bass_kernel_spmd(nc, in_maps, list(range(N_CORES)), trace=trace)
    context = np.concatenate(
        [np.asarray(r["context"], dtype=np.float32) for r in res.results], axis=0
    )
    weights = np.concatenate(
        [np.asarray(r["weights"], dtype=np.float32) for r in res.results], axis=0
    )
    return res, context, weights


def kernel(decoder_hidden, encoder_outputs, mask, W_enc, W_dec, v):
    _, context, weights = run(
        {
            "decoder_hidden": decoder_hidden,
            "encoder_outputs": encoder_outputs,
            "mask": mask,
            "W_enc": W_enc,
            "W_dec": W_dec,
            "v": v,
        }
    )
    return context, weights
